# revision 5
# baseline (speedup 1.0000x reference)
import sys

sys.path.insert(0, "/opt/trn_rl_repo")

import numpy as np

import concourse.bass as bass
import concourse.bacc as bacc
import concourse.mybir as mybir
import concourse.tile as tile
from concourse.bass_utils import run_bass_kernel_spmd

F32 = mybir.dt.float32
BF16 = mybir.dt.bfloat16
F8 = mybir.dt.float8e4

# Problem constants
B, C, Dn, Hn, Wn = 2, 64, 64, 64, 64
H2, HID = 256, 128
KT, DIL, PAD = 3, 3, 3
EPS = 1e-5
WSCALE = 8.0  # dw weights stored x8 in fp8; folded back in the gate copies

# Sharding: 8 cores = 2 samples x 4 D-chunks of 16 slices; halo 3 each side.
NCORES = 8
JD = 4
DCH = Dn // JD           # 16 own d slices per core
DHL = DCH + 2 * PAD      # 22 local (haloed) d slices
HB, HBS = 2, Hn // 2     # 2 h blocks of 32 own rows
HIN = HBS + PAD          # 35 input rows per block
RROWS = 40               # ring rows: 1 lead pad + 3 halo + 32 own + 3 halo + 1 tail
WP = 76                  # padded row pitch: 6 + 64 + 6
NSP = Dn * Hn * Wn
NTOT = C * NSP

# dw conv chunking: own rows per chunk (6,6,6,6,6,2), contiguous 456/152 windows
CHUNKS = [(0, 6), (6, 6), (12, 6), (18, 6), (24, 6), (30, 2)]
NCH = len(CHUNKS)

HALO_DLS = [0, 1, 2, DHL - 3, DHL - 2, DHL - 1]

_CACHED = {}


def _build_nc(use_collectives=True):
    nc = bacc.Bacc(None, num_devices=NCORES)

    x_ext = nc.declare_dram_parameter("x", [C, DHL, Hn, Wn], F32, isOutput=False)
    at_ext = nc.declare_dram_parameter("at", [C, H2], F32, isOutput=False)
    qrow_ext = nc.declare_dram_parameter("qrow", [1, H2], F32, isOutput=False)
    trow_ext = nc.declare_dram_parameter("trow", [1, H2], F32, isOutput=False)
    dwp_ext = nc.declare_dram_parameter("dwp", [128, 30 * 256], F8, isOutput=False)
    dwba_ext = nc.declare_dram_parameter("dwba", [128, 1], F32, isOutput=False)
    dwbb_ext = nc.declare_dram_parameter("dwbb", [128, 1], F32, isOutput=False)
    scaw_ext = nc.declare_dram_parameter("sca_wT", [HID, HID], BF16, isOutput=False)
    scab_ext = nc.declare_dram_parameter("sca_b", [HID, 1], F32, isOutput=False)
    postw_ext = nc.declare_dram_parameter("post_wT", [HID, C], F32, isOutput=False)
    pb_ext = nc.declare_dram_parameter("pb", [C, 1], F32, isOutput=False)
    mask_ext = nc.declare_dram_parameter("mask", [1, DHL], F32, isOutput=False)
    out_ext = nc.declare_dram_parameter("out", [C, DCH, Hn, Wn], F32, isOutput=True)

    groups = [[0, 1, 2, 3], [4, 5, 6, 7]]
    mm = mybir.AluOpType.mult
    aa = mybir.AluOpType.add

    with tile.TileContext(nc) as tc:
        with (
            tc.tile_pool(name="wts", bufs=1) as wp,
            tc.tile_pool(name="small", bufs=1) as sp,
            tc.tile_pool(name="dram", bufs=1, space="DRAM") as dp,
        ):
            # ---- persistent weight tiles ----
            at_t = wp.tile([C, H2], F32, tag="at")
            qrow_t = wp.tile([1, H2], F32, tag="qrow")
            trow_t = wp.tile([1, H2], F32, tag="trow")
            dwp_t = wp.tile([128, 30 * 256], F8, tag="dwp")
            dwba_t = wp.tile([128, 1], F32, tag="dwba")
            dwbb_t = wp.tile([128, 1], F32, tag="dwbb")
            scaw_t = wp.tile([HID, HID], BF16, tag="scaw")
            scab_t = wp.tile([HID, 1], F32, tag="scab")
            postw_t = wp.tile([HID, C], F32, tag="postw")
            pb_t = wp.tile([C, 1], F32, tag="pb")
            mask_t = wp.tile([1, DHL], F32, tag="mask")
            h3_all = wp.tile([HID, DCH * Hn * Wn], F8, tag="h3all")
            pool_cols = wp.tile([HID, HB * DCH * NCH], F32, tag="poolc")

            nc.sync.dma_start(at_t[:], at_ext[:])
            nc.sync.dma_start(qrow_t[:], qrow_ext[:])
            nc.sync.dma_start(trow_t[:], trow_ext[:])
            nc.sync.dma_start(dwp_t[:], dwp_ext[:])
            nc.sync.dma_start(dwba_t[:], dwba_ext[:])
            nc.sync.dma_start(dwbb_t[:], dwbb_ext[:])
            nc.sync.dma_start(scaw_t[:], scaw_ext[:])
            nc.sync.dma_start(scab_t[:], scab_ext[:])
            nc.sync.dma_start(postw_t[:], postw_ext[:])
            nc.sync.dma_start(pb_t[:], pb_ext[:])
            nc.sync.dma_start(mask_t[:], mask_ext[:])

            # ---- stage 1: GroupNorm stats over own region ----
            sum_cols = sp.tile([128, 8], F32, tag="sumc")
            sq_cols = sp.tile([128, 8], F32, tag="sqc")
            with tc.tile_pool(name="stats", bufs=3) as stp:
                for i in range(8):
                    xt = stp.tile([128, Hn * Wn], F32, tag="sx")
                    for k in range(2):
                        nc.sync.dma_start(
                            xt[64 * k : 64 * k + 64, :],
                            x_ext[:, PAD + 2 * i + k, :, :].rearrange(
                                "c h w -> c (h w)"
                            ),
                        )
                    scr = stp.tile([128, Hn * Wn], F32, tag="scr")
                    flat = xt[:]
                    nc.vector.tensor_reduce(
                        sum_cols[:, i : i + 1], flat, mybir.AxisListType.X, aa
                    )
                    nc.vector.tensor_mul(scr[:], flat, flat)
                    nc.vector.tensor_reduce(
                        sq_cols[:, i : i + 1], scr[:], mybir.AxisListType.X, aa
                    )

            both = sp.tile([128, 2], F32, tag="both")
            nc.vector.tensor_reduce(both[:, 0:1], sum_cols[:], mybir.AxisListType.X, aa)
            nc.vector.tensor_reduce(both[:, 1:2], sq_cols[:], mybir.AxisListType.X, aa)
            ones_c = sp.tile([128, 1], F32, tag="onesc")
            nc.gpsimd.memset(ones_c[:], 1.0)
            part = sp.tile([1, 2], F32, tag="part")
            with tc.tile_pool(name="stpsum", bufs=1, space="PSUM") as stps:
                pps_ = stps.tile([1, 2], F32, tag="stp")
                nc.tensor.matmul(pps_[:], ones_c[:], both[:])
                nc.vector.tensor_copy(part[:], pps_[:])

            st_in = dp.tile([1, 2], F32, tag="stin")
            st_out = dp.tile([1, 2], F32, tag="stout")
            nc.sync.dma_start(st_in[:], part[:])
            if use_collectives:
                nc.gpsimd.collective_compute(
                    "AllReduce", aa, replica_groups=groups,
                    ins=[st_in.opt()], outs=[st_out.opt()],
                )
            else:
                nc.sync.dma_start(st_out[:], st_in[:])
            tot = sp.tile([1, 2], F32, tag="tot")
            nc.sync.dma_start(tot[:], st_out[:])

            # ---- derive mu, r = rsqrt(var+eps); fold into pre-conv weights ----
            mu = sp.tile([1, 1], F32, tag="mu")
            e2 = sp.tile([1, 1], F32, tag="e2")
            nc.vector.tensor_scalar_mul(mu[:], tot[:, 0:1], 1.0 / NTOT)
            nc.vector.tensor_scalar_mul(e2[:], tot[:, 1:2], 1.0 / NTOT)
            mu2 = sp.tile([1, 1], F32, tag="mu2")
            nc.vector.tensor_mul(mu2[:], mu[:], mu[:])
            v = sp.tile([1, 1], F32, tag="v")
            nc.vector.tensor_sub(v[:], e2[:], mu2[:])
            nc.vector.tensor_scalar_add(v[:], v[:], EPS)
            sq = sp.tile([1, 1], F32, tag="sqv")
            nc.scalar.sqrt(sq[:], v[:])
            r0 = sp.tile([1, 1], F32, tag="r0")
            nc.vector.reciprocal(r0[:], sq[:])
            z = sp.tile([1, 1], F32, tag="z")
            nc.vector.tensor_mul(z[:], r0[:], r0[:])
            nc.vector.tensor_mul(z[:], z[:], v[:])
            nc.vector.tensor_scalar(z[:], z[:], -0.5, 1.5, mm, aa)
            r_ = sp.tile([1, 1], F32, tag="r_")
            nc.vector.tensor_mul(r_[:], r0[:], z[:])
            nrmu = sp.tile([1, 1], F32, tag="nrmu")
            nc.vector.tensor_mul(nrmu[:], r_[:], mu[:])
            nc.vector.tensor_scalar_mul(nrmu[:], nrmu[:], -1.0)

            ones_row = sp.tile([1, 128], F32, tag="onesr")
            nc.gpsimd.memset(ones_row[:], 1.0)
            r_b = sp.tile([C, 1], F32, tag="r_b")
            with tc.tile_pool(name="bcpsum", bufs=2, space="PSUM") as bcp:
                rbp = bcp.tile([C, 1], F32, tag="rbp")
                nc.tensor.matmul(rbp[:], ones_row[:, 0:C], r_[:])
                nc.vector.tensor_copy(r_b[:], rbp[:])

            # pre-conv lhsT [65, 256]: rows 0-63 = r*A^T, row 64 = q - r*mu*t
            lhsT_main = sp.tile([C + 1, H2], BF16, tag="lhsTm")
            nc.vector.tensor_scalar(lhsT_main[0:C, :], at_t[:], r_b[:], None, mm)
            nc.vector.scalar_tensor_tensor(
                lhsT_main[C : C + 1, :], trow_t[:], nrmu[:], qrow_t[:], mm, aa
            )
            lhsT_by_dl = {}
            with tc.tile_pool(name="mbpsum", bufs=2, space="PSUM") as mbp:
                for dl in range(DHL):
                    if dl in HALO_DLS:
                        mbps = mbp.tile([C + 1, 1], F32, tag="mbps")
                        nc.tensor.matmul(
                            mbps[:], ones_row[:, 0 : C + 1], mask_t[:, dl : dl + 1]
                        )
                        mb = sp.tile([C + 1, 1], F32, tag=f"mb{dl}")
                        nc.vector.tensor_copy(mb[:], mbps[:])
                        lv = sp.tile([C + 1, H2], BF16, tag=f"lv{dl}")
                        nc.vector.tensor_scalar(lv[:], lhsT_main[:], mb[:], None, mm)
                        lhsT_by_dl[dl] = lv
                    else:
                        lhsT_by_dl[dl] = lhsT_main

            # ---- stage 2: pre-conv -> fp8 DoubleRow dw conv -> gate -> pool ----
            with (
                tc.tile_pool(name="xin", bufs=3) as xp,
                tc.tile_pool(name="ring", bufs=8) as rp,
                tc.tile_pool(name="gate", bufs=4) as gp_,
                tc.tile_pool(name="prepsum", bufs=3, space="PSUM") as pps,
                tc.tile_pool(name="dwpsum", bufs=4, space="PSUM") as dps,
            ):
                for hb in range(HB):
                    rs = 4 if hb == 0 else 1  # ring row of first loaded x row
                    xh0 = 0 if hb == 0 else HBS - PAD
                    ring = {}
                    for dl in range(DHL):
                        # load x [64, 35, 64] + ones row
                        xt = xp.tile([C, HIN, Wn], F32, tag="xt")
                        nc.sync.dma_start(xt[:, :, :], x_ext[:, dl, xh0 : xh0 + HIN, :])
                        xb = xp.tile([C + 1, HIN, Wn], BF16, tag="xb")
                        if dl % 2 == 0:
                            nc.vector.tensor_copy(xb[0:C, :, :], xt[:])
                        else:
                            nc.scalar.copy(xb[0:C, :, :], xt[:])
                        nc.gpsimd.memset(xb[C : C + 1, :, :], 1.0)

                        # fp8 ring tile [128, 40, 76] per group
                        t_g = []
                        for g in range(2):
                            t3 = rp.tile([128, RROWS, WP], F8, tag=f"ring{g}")
                            t3a = t3[:]
                            # W pads rows 0..38 (incl spill into next row's left pad)
                            wpad = bass.AP(
                                tensor=t3a.tensor, offset=t3a.offset + 70,
                                ap=[t3a.ap[0], [WP, RROWS - 1], [1, 12]],
                            )
                            nc.gpsimd.memset(wpad, 0.0)
                            if hb == 0:
                                nc.gpsimd.memset(t3[:, 0:4, :], 0.0)
                                nc.gpsimd.memset(t3[:, RROWS - 1 :, :], 0.0)
                            else:
                                nc.gpsimd.memset(t3[:, 0:1, :], 0.0)
                                nc.gpsimd.memset(t3[:, RROWS - 4 :, :], 0.0)
                            t_g.append(t3)

                        lhsT = lhsT_by_dl[dl]
                        rows_per_chunk = [8, 8, 8, 8, 3]
                        rc0 = 0
                        for ci, nr in enumerate(rows_per_chunk):
                            rhs = xb[:, rc0 : rc0 + nr, :]
                            for g in range(2):
                                ps = pps.tile([128, 512], F32, tag="pps")
                                nc.tensor.matmul(
                                    ps[:, : nr * Wn], lhsT[:, g * 128 : (g + 1) * 128], rhs
                                )
                                dest = t_g[g][:, rs + rc0 : rs + rc0 + nr, 6:70]
                                if (ci + g) % 2 == 0:
                                    nc.scalar.copy(dest, ps[:, : nr * Wn])
                                else:
                                    nc.vector.tensor_copy(dest, ps[:, : nr * Wn])
                            rc0 += nr

                        ring[dl] = t_g

                        if dl >= 6:
                            dl0 = dl - 6  # own-d index 0..15
                            for R, nrr in CHUNKS:
                                L = nrr * WP
                                gps = []
                                for g in range(2):
                                    ps = dps.tile([128, 456], F32, tag="dps")
                                    for tz in range(3):
                                        src = ring[dl + (tz - 2) * 3][g][:]
                                        for j in range(5):
                                            blk = ((g * 3 + tz) * 5 + j) * 256
                                            lw = dwp_t[:, blk : blk + 256].rearrange(
                                                "p (two m) -> p two m", two=2
                                            )
                                            if j < 3:
                                                st = (R + 4 + 3 * (j - 1)) * WP - 3
                                                dlt = 6
                                            elif j == 3:
                                                st = (R + 1) * WP
                                                dlt = 6 * WP
                                            else:
                                                st = (R + 4) * WP
                                                dlt = WP
                                            rhs = bass.AP(
                                                tensor=src.tensor,
                                                offset=src.offset + st,
                                                ap=[src.ap[0], [dlt, 2], [1, L]],
                                            )
                                            nc.tensor.matmul(
                                                ps[:, :L], lw, rhs,
                                                start=(tz == 0 and j == 0),
                                                stop=(tz == 2 and j == 4),
                                                perf_mode=mybir.MatmulPerfMode.DoubleRow,
                                            )
                                    gps.append(ps)
                                # valid columns view of psum: [128, nrr, 64]
                                def vslice(ps):
                                    pa = ps[:]
                                    return bass.AP(
                                        tensor=pa.tensor, offset=pa.offset + 6,
                                        ap=[pa.ap[0], [WP, nrr], [1, 64]],
                                    )
                                a_sb = gp_.tile([128, 6 * 64], F32, tag="asb")
                                a3 = a_sb[:, : nrr * 64].rearrange(
                                    "p (a b) -> p a b", b=64
                                )
                                nc.scalar.activation(
                                    a3, vslice(gps[0]),
                                    mybir.ActivationFunctionType.Identity,
                                    bias=dwba_t[:], scale=1.0 / (WSCALE * WSCALE),
                                )
                                cidx = ((hb * DCH + dl0) * NCH) + CHUNKS.index((R, nrr))
                                h0g = dl0 * Hn * Wn + (hb * HBS + R) * Wn
                                nc.vector.scalar_tensor_tensor(
                                    h3_all[:, h0g : h0g + nrr * 64].rearrange(
                                        "p (a b) -> p a b", b=64
                                    ),
                                    vslice(gps[1]), dwbb_t[:], a3,
                                    aa, mm,
                                    accum_out=pool_cols[:, cidx : cidx + 1],
                                )

            # ---- stage 3: SCA pool allreduce -> attn -> fold into post weights ----
            pool_p = sp.tile([HID, 1], F32, tag="poolp")
            nc.vector.tensor_reduce(pool_p[:], pool_cols[:], mybir.AxisListType.X, aa)
            pl_in = dp.tile([HID, 1], F32, tag="plin")
            pl_out = dp.tile([HID, 1], F32, tag="plout")
            nc.sync.dma_start(pl_in[:], pool_p[:])
            if use_collectives:
                nc.gpsimd.collective_compute(
                    "AllReduce", aa, replica_groups=groups,
                    ins=[pl_in.opt()], outs=[pl_out.opt()],
                )
            else:
                nc.sync.dma_start(pl_out[:], pl_in[:])
            pool_f = sp.tile([HID, 1], F32, tag="poolf")
            nc.sync.dma_start(pool_f[:], pl_out[:])
            pool_bf = sp.tile([HID, 1], BF16, tag="poolbf")
            nc.vector.tensor_copy(pool_bf[:], pool_f[:])

            attn = sp.tile([HID, 1], F32, tag="attn")
            with tc.tile_pool(name="scapsum", bufs=1, space="PSUM") as scp:
                aps = scp.tile([HID, 1], F32, tag="aps")
                nc.tensor.matmul(aps[:], scaw_t[:], pool_bf[:])
                nc.scalar.activation(
                    attn[:], aps[:], mybir.ActivationFunctionType.Identity,
                    bias=scab_t[:], scale=1.0,
                )
            post_lhsT = sp.tile([HID, C], BF16, tag="postl")
            nc.vector.tensor_scalar(post_lhsT[:], postw_t[:], attn[:], None, mm)

            # ---- stage 4: post-conv + bias + residual ----
            with (
                tc.tile_pool(name="xres", bufs=3) as xrp,
                tc.tile_pool(name="outp", bufs=3) as op_,
                tc.tile_pool(name="postpsum", bufs=2, space="PSUM") as ppo,
            ):
                for dl0 in range(DCH):
                    xr = xrp.tile([C, Hn, Wn], F32, tag="xr")
                    nc.sync.dma_start(xr[:], x_ext[:, PAD + dl0, :, :])
                    ot = op_.tile([C, Hn * Wn], F32, tag="ot")
                    xrf = xr[:].rearrange("p a b -> p (a b)")
                    for half in range(2):
                        ps = ppo.tile([C, 2048], F32, tag="ppo")
                        for q4 in range(4):
                            c0 = half * 2048 + q4 * 512
                            nc.tensor.matmul(
                                ps[:, q4 * 512 : (q4 + 1) * 512],
                                post_lhsT[:],
                                h3_all[:, dl0 * Hn * Wn + c0 : dl0 * Hn * Wn + c0 + 512],
                            )
                        nc.vector.scalar_tensor_tensor(
                            ot[:, half * 2048 : (half + 1) * 2048],
                            ps[:], pb_t[:], xrf[:, half * 2048 : (half + 1) * 2048],
                            aa, aa,
                        )
                    nc.sync.dma_start(
                        out_ext[:, dl0, :, :],
                        ot[:].rearrange("p (a b) -> p a b", b=Wn),
                    )

    nc.finalize()
    return nc


def _host_prep(inputs):
    import ml_dtypes

    x = np.asarray(inputs["x"], np.float32)
    gam = np.asarray(inputs["gn_gamma"], np.float32)
    bet = np.asarray(inputs["gn_beta"], np.float32)
    pre_w = np.asarray(inputs["pre_w"], np.float32)
    pre_b = np.asarray(inputs["pre_b"], np.float32)
    ddc_w = np.asarray(inputs["ddc_w"], np.float32).reshape(H2, KT, KT, KT)
    ddc_b = np.asarray(inputs["ddc_b"], np.float32)
    sca_w = np.asarray(inputs["sca_w"], np.float32)
    sca_b = np.asarray(inputs["sca_b"], np.float32)
    post_w = np.asarray(inputs["post_w"], np.float32)
    post_b = np.asarray(inputs["post_b"], np.float32)

    A = pre_w * gam[None, :]                    # [256, 64]
    at = np.ascontiguousarray(A.T)              # [64, 256]
    qrow = (pre_b + pre_w @ bet)[None, :]       # [1, 256]
    trow = A.sum(axis=1)[None, :]               # [1, 256]

    # dw pair weights: [128, g(2) x tz(3) x j(5) x two(2) x 128] fp8, x WSCALE
    dwp = np.zeros((128, 2, 3, 5, 2, 128), np.float32)
    idx = np.arange(128)
    for g in range(2):
        ch = ddc_w[g * 128 : (g + 1) * 128] * WSCALE  # [128,3,3,3]
        for tz in range(3):
            for j in range(5):
                if j < 3:
                    taps = [(tz, j, 0), (tz, j, 2)]
                elif j == 3:
                    taps = [(tz, 0, 1), (tz, 2, 1)]
                else:
                    taps = [(tz, 1, 1), None]
                for i, tp in enumerate(taps):
                    if tp is None:
                        continue
                    dwp[idx, g, tp[0], j, i, idx] = ch[idx, tp[0], tp[1], tp[2]]
    dwp = dwp.reshape(128, 30 * 256)

    bf = lambda a: a.astype(ml_dtypes.bfloat16)
    f8 = lambda a: a.astype(ml_dtypes.float8_e4m3)
    common = {
        "at": at,
        "qrow": qrow,
        "trow": trow,
        "dwp": f8(dwp),
        "dwba": (ddc_b[0:HID] / WSCALE)[:, None].astype(np.float32),
        "dwbb": (ddc_b[HID:] * WSCALE)[:, None].astype(np.float32),
        "sca_wT": bf(np.ascontiguousarray((sca_w / NSP).T)),
        "sca_b": sca_b[:, None].astype(np.float32),
        "post_wT": np.ascontiguousarray(post_w.T).astype(np.float32),
        "pb": post_b[:, None].astype(np.float32),
    }

    in_maps = []
    for core in range(NCORES):
        b, j = core // JD, core % JD
        lo, hi = j * DCH - PAD, j * DCH + DCH + PAD
        xs = np.zeros((C, DHL, Hn, Wn), np.float32)
        clo, chi = max(lo, 0), min(hi, Dn)
        xs[:, clo - lo : chi - lo] = x[b, :, clo:chi]
        mask = np.ones((1, DHL), np.float32)
        for dl in range(DHL):
            dg = lo + dl
            if dg < 0 or dg >= Dn:
                mask[0, dl] = 0.0
        m = dict(common)
        m["x"] = xs
        m["mask"] = mask
        in_maps.append(m)
    return in_maps


def kernel(**inputs):
    if "nc" not in _CACHED:
        _CACHED["nc"] = _build_nc()
    nc = _CACHED["nc"]
    in_maps = _host_prep(inputs)
    res = run_bass_kernel_spmd(nc, in_maps, list(range(NCORES)))
    out = np.zeros((B, C, Dn, Hn, Wn), np.float32)
    for core in range(NCORES):
        b, j = core // JD, core % JD
        out[b, :, j * DCH : (j + 1) * DCH] = np.asarray(res.results[core]["out"])
    return out
